# revision 59
# baseline (speedup 1.0000x reference)
"""Multi-head attention forward for nn_AttentionStoreActivationPrune.

The reference's straight-through pattern ``sg(dense) + prune - sg(prune)``
is numerically ``dense`` in the forward pass, so every top-k masking branch
cancels and the output equals a plain multi-head attention forward.

Sharding: data-parallel over batch - 8 batch elements, one per NeuronCore.

Numerics (cost model: matmul time = out_free_rows * cycles; fp8 DoubleRow
= 0.5 cycles/row vs 1.0 for bf16/f32r):
  - QKV projections run as THREE error-compensated fp8e4m3 DoubleRow terms
      X@W ~= X8@W8 + (X8/16)@dW8 + (dX8/16)@W8s
    with W prescaled by 32 (keeps fp8 normals; 56% of raw W entries would
    be subnormal), dW8 = f8(16*(32W - f8(32W))), W8s = f8(2W), dX8 =
    f8(16*(X - f8(X))).  Each term contracts 256/instruction (ko-paired
    DoubleRow): a 768-deep projection costs 4.5 cyc/col vs 6.0 bf16 at
    ~bf16 accuracy.
  - Scores use the zero-slot DoubleRow trick: lhsT = K8 pairs, rhs = Q8
    pairs with slot1 zeroed -> 0.5 cyc/row at contraction 64 (the unused
    slot multiplies zero).  Q/K evict to fp8: the only lossy eviction
    (~1.3% of the 2e-2 gate on its own; fp8 anywhere else fails - measured
    per site).
  - exp on the scalar engine is the second bottleneck (~40us busy); one
    [128, 579] call per (head, s_k-chunk), bf16 out.  The softmax
    denominator rides along as a 65th V column (4.0, folding part of the
    1/32 prescale); s_k pad rows carry 0 there so exp(0)=1 pad terms drop.
  - ctx is sequence-major (65-row moving dim) so the denominator is
    per-(partition, block) and normalization fuses into the PSUM eviction
    via a broadcast reciprocal multiply (bf16).  ctx is then PE-transposed
    (identity matmul) per head-pair to feature-major for the bf16 output
    projection; the residual 1/8 scale folds into the final eviction.

Scheduling (engines execute their static schedule in order):
  - The attention stream is pipelined one head deep: head h emits only
    scores+exp; its ctx block (zeroing dummy, 25 accumulating matmuls,
    reciprocal + normalize eviction) is emitted two heads later, so a
    late ctx accumulator can never stall the score/exp stream feeding the
    ACT engine.
  - QK/V projection chunks (9-matmul, 1-bank PSUM groups) and the ctx
    transposes interleave as PE filler between score chunks.  PSUM banks:
    scores 2x2, projections 2x1, ctx accumulators 2x1 = 8.
  - Inputs arrive as a few large packed DMAs (HWDGE issue is ~630ns each,
    serialized); order: x8, wqk0, xres, wv8, wqk1, wvres, wqk2..5, wo.
  - Every PSUM accumulation group starts with a single instruction
    covering its whole 2KB bank region (chunk starts are bank-aligned;
    the ctx accumulator uses an explicit zeroing matmul), which is valid
    under both per-instruction and zero-region PSUM-start semantics.

Biases are structurally zero in this problem (setup_inputs fills zeros);
kernel() checks and falls back to a with-bias program built on demand.
"""

import numpy as np
import ml_dtypes

S, H, NH, HD, KO = 577, 768, 12, 64, 6
B = 8
SQP = 579           # padded s_q: 3 * 193 (DoubleRow moving chunks)
SKP = 640           # padded s_k / x8 free size: 5 * 128
NQ8 = 193           # score / qk-projection moving chunk
SCH = [(0, 128), (128, 128), (256, 128), (384, 128), (512, 65)]
EXP_SCALE = 1.0 / 8192.0   # (1/8) / (32*32) : exp(scores/8) with 32x q,k

_CACHE = {}

F8 = ml_dtypes.float8_e4m3
BF16 = ml_dtypes.bfloat16


KNOBS = {"warms": 8, "epool": 24, "dma_order": "B"}


def _build_nc(zero_bias):
    import concourse.mybir as mybir
    import concourse.tile as tile
    from concourse import bacc

    f32 = mybir.dt.float32
    f8 = mybir.dt.float8e4
    bf16 = mybir.dt.bfloat16
    ADD = mybir.AluOpType.add
    MUL = mybir.AluOpType.mult
    EXP = mybir.ActivationFunctionType.Exp
    CPY = mybir.ActivationFunctionType.Copy
    DR = mybir.MatmulPerfMode.DoubleRow

    nc = bacc.Bacc("TRN2", target_bir_lowering=False, debug=False)

    x8_d = nc.dram_tensor("x8", [128, KO, SKP], f8, kind="ExternalInput")
    xres_d = nc.dram_tensor("xres", [128, KO, 2, SKP], f8,
                            kind="ExternalInput")
    # packed per-column-block QK weight triples: [128, ko, {q,k}, term, 128]
    wqk_d = [nc.dram_tensor(f"wqk{b}", [128, KO, 2, 3, 128], f8,
                            kind="ExternalInput") for b in range(KO)]
    wv8_d = nc.dram_tensor("wv8", [128, KO, H], f8, kind="ExternalInput")
    wvres_d = nc.dram_tensor("wvres", [128, KO, 2, H], f8,
                             kind="ExternalInput")
    wo_d = nc.dram_tensor("wo", [128, KO, H], bf16, kind="ExternalInput")
    id_d = nc.dram_tensor("ident", [128, 128], bf16, kind="ExternalInput")
    if not zero_bias:
        bq_d = nc.dram_tensor("bq32", [H], f32, kind="ExternalInput")
        bk_d = nc.dram_tensor("bk32", [H], f32, kind="ExternalInput")
        bv_d = nc.dram_tensor("bv32", [1, H], bf16, kind="ExternalInput")
        bo_d = nc.dram_tensor("bo8", [1, H], bf16, kind="ExternalInput")
        ones_d = nc.dram_tensor("ones", [1, 128], bf16, kind="ExternalInput")
    out_d = nc.dram_tensor("out", [S, H], bf16, kind="ExternalOutput")

    with tile.TileContext(nc) as tc:
        with tc.tile_pool(name="consts", bufs=1) as consts, \
             tc.tile_pool(name="wts", bufs=1) as wts, \
             tc.tile_pool(name="bigs", bufs=1) as bigs, \
             tc.tile_pool(name="epool", bufs=KNOBS['epool']) as epool, \
             tc.tile_pool(name="mid", bufs=4) as mid, \
             tc.tile_pool(name="outs", bufs=3) as outsp:

            ident = consts.tile([128, 128], bf16, tag="ident")
            nc.scalar.dma_start(out=ident, in_=id_d[:])
            warm = consts.tile([128, 2, 256], f8, tag="warm")
            nc.vector.memset(warm[:, :, :].bitcast(f32), 0.0)
            if not zero_bias:
                ones = consts.tile([1, 128], bf16, tag="ones")
                nc.scalar.dma_start(out=ones, in_=ones_d[:])
                bq_t = consts.tile([128, KO], f32, tag="bq")
                nc.scalar.dma_start(
                    out=bq_t, in_=bq_d.rearrange("(ko ki) -> ki ko", ki=128))
                bk_t = consts.tile([128, KO], f32, tag="bk")
                nc.scalar.dma_start(
                    out=bk_t, in_=bk_d.rearrange("(ko ki) -> ki ko", ki=128))
                bv_t = consts.tile([1, H], bf16, tag="bv")
                nc.scalar.dma_start(out=bv_t, in_=bv_d[:])
                bo_t = consts.tile([1, H], bf16, tag="bo")
                nc.scalar.dma_start(out=bo_t, in_=bo_d[:])

            # ---- big activation tiles ----
            X8 = bigs.tile([128, KO, SKP], f8, tag="X8")
            XRES = bigs.tile([128, KO, 2, SKP], f8, tag="XRES")
            QT8 = [bigs.tile([128, 2, SKP], f8, tag=f"QT{i}", name=f"QT{i}")
                   for i in range(KO)]
            KT8 = [bigs.tile([128, 2, SKP], f8, tag=f"KT{i}", name=f"KT{i}")
                   for i in range(KO)]
            # zero the pair tiles (slot1 must be 0 for the zero-slot trick;
            # K slot0 cols >= S must be 0 so padded score rows exp to finite)
            for t in QT8 + KT8:
                nc.gpsimd.memset(t[:, :, :].bitcast(f32), 0.0)
            Vaug = [bigs.tile([128, NH, HD + 1], bf16, tag=f"vaug{i}",
                              name=f"vaug{i}")
                    for i in range(len(SCH))]
            for sc, (s0, sz) in enumerate(SCH):
                # denominator ride-along column (4.0 folds part of the 32x V
                # prescale).  Pad rows (s_k >= 577, where E = exp(0) = 1)
                # must carry 0 so they don't inflate the denominator.
                if sz < 128:
                    nc.vector.memset(Vaug[sc][64:128, :, HD:HD + 1], 0.0)
                nc.vector.memset(Vaug[sc][0:sz, :, HD:HD + 1], 4.0)
            CTXN = bigs.tile([128, 5, NH, HD], bf16, tag="CTXN")
            CTXT = bigs.tile([128, KO, SKP], bf16, tag="CTXT")

            wqk_t = [wts.tile([128, KO, 2, 3, 128], f8, tag=f"wqk{b}",
                              name=f"wqk{b}") for b in range(KO)]
            wv8_t = wts.tile([128, KO, H], f8, tag="wv8")
            wvres_t = wts.tile([128, KO, 2, H], f8, tag="wvres")
            wo_t = wts.tile([128, KO, H], bf16, tag="wo")

            # ---- input DMAs, few and large, in consumption order ----
            dma_specs = {
                "A": ["x8", "wqk0", "xres", "wv8", "wvres", "wqk1", "wqk2",
                      "wqk3", "wqk4", "wqk5", "wo"],
                "B": ["x8", "wqk0", "xres", "wv8", "wqk1", "wvres", "wqk2",
                      "wqk3", "wqk4", "wqk5", "wo"],
                "D": ["x8", "wqk0", "xres", "wvres", "wv8", "wqk1", "wqk2",
                      "wqk3", "wqk4", "wqk5", "wo"],
                "E": ["x8", "wqk0", "xres", "wv8", "wvres", "wqk1", "wqk2",
                      "wqk3", "wqk4", "wqk5", "wo"],
                "C": ["x8", "wqk0", "xres", "wvres", "wv8", "wqk1", "wqk2",
                      "wqk3", "wqk4", "wqk5", "wo"],
            }[KNOBS["dma_order"]]
            dma_map = {"x8": (X8, x8_d), "xres": (XRES, xres_d),
                       "wv8": (wv8_t, wv8_d), "wvres": (wvres_t, wvres_d),
                       "wo": (wo_t, wo_d)}
            for b in range(KO):
                dma_map[f"wqk{b}"] = (wqk_t[b], wqk_d[b])
            for nm in dma_specs:
                dst, srcd = dma_map[nm]
                nc.sync.dma_start(out=dst, in_=srcd[:])

            # X-side operand per compensation term
            def xop(t, ko, lo, hi):
                if t == 0:
                    return X8[:, 2 * ko:2 * ko + 2, lo:hi]
                return XRES[:, 2 * ko:2 * ko + 2, t - 1, lo:hi]

            pscore_cm = tc.tile_pool(name="pscore", bufs=2, space="PSUM")
            pproj_cm = tc.tile_pool(name="pproj", bufs=2, space="PSUM")
            pctx_cm = tc.tile_pool(name="pctx", bufs=2, space="PSUM")
            pscore = pscore_cm.__enter__()
            pproj = pproj_cm.__enter__()
            pctx = pctx_cm.__enter__()

            for wi in range(KNOBS['warms']):
                pw = pproj.tile([128, 512], f32, tag="pp",
                                name=f"warm{wi}")
                nc.tensor.matmul(pw[:, 0:256], warm[:, :, 0:128],
                                 warm[:, :, :], start=True, stop=True,
                                 perf_mode=DR)

            def qk_chunk(koh, iw, qc):
                """One projection moving-chunk: a 9-matmul 1-bank PSUM
                accumulation group plus its fp8 eviction (koh 0 evicts on
                the scalar engine, idle before the first exp)."""
                pq = pproj.tile([128, 512], f32, tag="pp",
                                name=f"pq_{iw}_{koh}_{qc}")
                for it in range(3):
                    for ko in range(3):
                        nc.tensor.matmul(
                            pq[:, 0:NQ8],
                            wqk_t[koh][:, 2 * ko:2 * ko + 2, iw, it, :],
                            xop(it, ko, qc * NQ8, (qc + 1) * NQ8),
                            start=(it == 0 and ko == 0),
                            stop=(it == 2 and ko == 2),
                            perf_mode=DR,
                        )
                dst = (QT8, KT8)[iw][koh]
                dst_v = dst[:, 0, qc * NQ8:(qc + 1) * NQ8]
                if zero_bias:
                    if koh == 0 and iw == 0:
                        # Q evictions on the (idle) scalar engine, K on DVE:
                        # the two halves run in parallel ahead of the first
                        # score matmuls
                        nc.scalar.activation(out=dst_v, in_=pq[:, 0:NQ8],
                                             func=CPY)
                    else:
                        nc.vector.tensor_copy(out=dst_v, in_=pq[:, 0:NQ8])
                else:
                    bias_t = bq_t if iw == 0 else bk_t
                    nc.vector.tensor_scalar(
                        dst_v, pq[:, 0:NQ8],
                        bias_t[:, koh:koh + 1], None, ADD)

            def project_qk(koh):
                for iw in range(2):
                    for qc in range(3):
                        qk_chunk(koh, iw, qc)

            def v_chunk(sc, vc):
                s0, sz = SCH[sc]
                pv = pproj.tile([128, 512], f32, tag="pp",
                                name=f"pv{sc}_{vc}")

                def vop(t, ko):
                    if t == 0:
                        return wv8_t[:, 2 * ko:2 * ko + 2, :]
                    return wvres_t[:, 2 * ko:2 * ko + 2, t - 1, :]

                for it in range(3):
                    for ko in range(3):
                        nc.tensor.matmul(
                            pv[:, 0:256],
                            xop(it, ko, s0, s0 + 128),
                            vop(it, ko)[..., vc * 256:(vc + 1) * 256],
                            start=(it == 0 and ko == 0),
                            stop=(it == 2 and ko == 2 and zero_bias),
                            perf_mode=DR,
                        )
                if not zero_bias:
                    nc.tensor.matmul(
                        pv[:, 0:256],
                        ones[0:1, 0:128],
                        bv_t[0:1, vc * 256:(vc + 1) * 256],
                        start=False, stop=True,
                    )
                nc.vector.tensor_copy(
                    out=Vaug[sc][:, 4 * vc:4 * vc + 4, 0:HD],
                    in_=pv[:, 0:256].rearrange("p (h d) -> p h d", d=HD))

            def transpose_koh(koh):
                """Transpose heads 2koh,2koh+1 of CTXN into CTXT row-block
                koh: 5 PE transposes + one DVE eviction (pproj pool, bf16
                fits in the same 1-bank footprint)."""
                ptr_f = pproj.tile([128, 512], f32, tag="pp",
                                   name=f"ptr{koh}")
                ptr = ptr_f[:, 0:320].bitcast(bf16).rearrange(
                    "p (a b) -> p a b", b=128)
                for qb in range(5):
                    nc.tensor.transpose(
                        ptr[:, qb, :],
                        CTXN[:, qb, 2 * koh:2 * koh + 2, :], ident)
                nc.vector.tensor_copy(
                    out=CTXT[:, koh, :],
                    in_=ptr[:, :, :].rearrange("p a b -> p (a b)"))

            def emit_scores(h, sc):
                koh, kb = h // 2, (h % 2) * HD
                ps = pscore.tile([128, 3, 256], f32, tag="ps",
                                 name=f"ps{h}_{sc}")
                for qc in range(3):
                    nc.tensor.matmul(
                        ps[:, qc, 0:NQ8],
                        KT8[koh][kb:kb + HD, :,
                                 sc * 128:sc * 128 + 128],
                        QT8[koh][kb:kb + HD, :,
                                 qc * NQ8:(qc + 1) * NQ8],
                        start=True, stop=True, perf_mode=DR,
                    )
                return ps

            def emit_exp(h, sc, ps):
                E = epool.tile([128, SKP], bf16, tag="e",
                               name=f"e{h}_{sc}")
                nc.scalar.activation(
                    out=E[:, 0:SQP].rearrange("p (c b) -> p c b", b=NQ8),
                    in_=ps[:, :, 0:NQ8],
                    func=EXP, scale=EXP_SCALE,
                )
                return E

            def emit_ctx(h, sc, E, pcs):
                for qb in range(5):
                    nc.tensor.matmul(
                        pcs[:, qb, :],
                        E[:, qb * 128:qb * 128 + 128],
                        Vaug[sc][:, h, :],
                        start=False,
                        stop=(sc == 4 and qb == 4),
                    )

            def ctx_dummy(h, pcs):
                # single start instruction zeroing the whole accumulator bank
                nc.tensor.matmul(
                    pcs[:, :, :].rearrange("p a b -> p (a b)"),
                    warm[:, 0, 0:128], X8[:, 0, 0:5 * (HD + 1)],
                    start=True, stop=False,
                )

            def finish_head(h, pcs):
                rec = mid.tile([128, 5], f32, tag="rec", name=f"rec{h}")
                nc.vector.reciprocal(out=rec, in_=pcs[:, :, HD:HD + 1])
                nc.vector.tensor_tensor(
                    out=CTXN[:, :, h, :],
                    in0=pcs[:, :, 0:HD],
                    in1=rec[:, :, None].broadcast_to([128, 5, HD]),
                    op=MUL,
                )

            def ctx_block(h, E):
                """Deferred ctx accumulation for an early head whose
                scores/exp ran ahead of the V projection."""
                pcs = pctx.tile([128, 5, HD + 1], f32, tag="pc",
                                name=f"pc{h}")
                ctx_dummy(h, pcs)
                for sc in range(5):
                    emit_ctx(h, sc, E[sc], pcs)
                finish_head(h, pcs)

            def attend_last(h, fillers):
                """Final head: inline ctx (chasing exp by two chunks) -
                there are no later scores to stall, and this shortens the
                post-attention serial chain into the output projection."""
                E = {}
                E[0] = emit_exp(h, 0, emit_scores(h, 0))
                E[1] = emit_exp(h, 1, emit_scores(h, 1))
                if fillers:
                    fillers.pop(0)()
                pcs = pctx.tile([128, 5, HD + 1], f32, tag="pc",
                                name=f"pc{h}")
                ctx_dummy(h, pcs)
                for sc in range(5):
                    if fillers:
                        fillers.pop(0)()
                    if sc + 2 < 5:
                        E[sc + 2] = emit_exp(
                            h, sc + 2, emit_scores(h, sc + 2))
                    emit_ctx(h, sc, E[sc], pcs)
                finish_head(h, pcs)

            def attend(h, fillers):
                """Scores/exp for head h only; ctx is emitted one head
                later (ctx_block) so a late ctx accumulator can never stall
                the score/exp stream in the in-order PE queue.  Fillers
                (QK/V projection chunks, transposes, the previous head's
                ctx block) are emitted between the score chunks."""
                E = {}
                for sc in range(5):
                    if sc >= 2 and fillers:
                        fillers.pop(0)()
                    E[sc] = emit_exp(h, sc, emit_scores(h, sc))
                return E

            # ---- phase A: first QK projection only; V-projection blocks
            # and later QK projections interleave into attention as filler
            # (pool buffer requests rotate with the score tiles) ----
            project_qk(0)

            # ---- phase B: attention pipelined one head deep: head h's
            # scores/exps stream while head h-1's ctx accumulates; V blocks
            # and the next QK projection's chunks fill the PE between score
            # chunks; transposes chase the ctx blocks ----
            Es = {}
            for h in range(NH):
                koh = h // 2
                fillers = []
                if h in (2, 4, 6):
                    # V evictions spread by deadline: head group vc is read
                    # by ctx_block(4*vc .. 4*vc+3), the first of which is
                    # emitted at h = 4*vc + 2 (two-head ctx lag)
                    vc = (h - 2) // 2
                    for sc in range(len(SCH)):
                        fillers.append(
                            lambda s=sc, v=vc: v_chunk(s, v))
                if h >= 2:
                    fillers.append(lambda hh=h - 2: ctx_block(hh, Es.pop(hh)))
                if h % 2 == 0 and koh + 1 < KO:
                    for iw in range(2):
                        for qc in range(3):
                            fillers.append(
                                lambda k=koh + 1, i=iw, q=qc:
                                qk_chunk(k, i, q))
                if h >= 5 and h % 2 == 1:
                    # transpose_koh(k) needs ctx_block(2k+1), emitted at
                    # h = 2k+3; schedule it at h = 2k+5
                    fillers.append(lambda k=(h - 5) // 2: transpose_koh(k))
                if h == NH - 1:
                    fillers.append(
                        lambda: ctx_block(NH - 2, Es.pop(NH - 2)))
                if h == NH - 1:
                    attend_last(h, fillers)
                else:
                    Es[h] = attend(h, fillers)
                for f in fillers:
                    f()
            for k in (KO - 2, KO - 1):
                transpose_koh(k)

            for p in (pctx_cm, pproj_cm, pscore_cm):
                p.__exit__(None, None, None)

            # ---- phase C: output projection (bf16), osb evict on ACT ----
            with tc.tile_pool(name="pout", bufs=2, space="PSUM") as pout:
                for sc, (s0, sz) in enumerate(SCH):
                    po = pout.tile([128, H], f32, tag="po", name=f"po{sc}")
                    for ko in range(KO):
                        for oc, (n0, nn) in enumerate(((0, 512), (512, 256))):
                            nc.tensor.matmul(
                                po[:, n0:n0 + nn],
                                CTXT[:, ko, s0:s0 + 128],
                                wo_t[:, ko, n0:n0 + nn],
                                start=(ko == 0),
                                stop=(ko == KO - 1 and oc == 1 and zero_bias),
                            )
                    if not zero_bias:
                        for oc, (n0, nn) in enumerate(((0, 512), (512, 256))):
                            nc.tensor.matmul(
                                po[:, n0:n0 + nn],
                                ones[0:1, 0:128],
                                bo_t[0:1, n0:n0 + nn],
                                start=False, stop=(oc == 1),
                            )
                    osb = outsp.tile([128, H], bf16, tag="osb")
                    nc.scalar.activation(out=osb, in_=po, func=CPY,
                                         scale=0.125)
                    deng = nc.sync if sc % 2 == 0 else nc.scalar
                    deng.dma_start(out=out_d[s0:s0 + sz, :],
                                   in_=osb[0:sz, :])

    nc.finalize()
    return nc


def _prep(hidden_states, Wq, Wk, Wv, Wo):
    """Host-side prep: fp8 triples for X and the QKV weights, bf16 Wo."""
    f8 = lambda a: np.asarray(a, F8)
    hs = np.ascontiguousarray(hidden_states, np.float32)
    xt = np.zeros((B, 128, KO, SKP), np.float32)
    xt[:, :, :, :S] = hs.transpose(0, 2, 1).reshape(B, KO, 128, S) \
        .transpose(0, 2, 1, 3)
    x8 = f8(xt)
    xres = np.empty((B, 128, KO, 2, SKP), F8)
    xres[:, :, :, 0, :] = f8(xt / 16.0)
    xres[:, :, :, 1, :] = f8(16.0 * (xt - x8.astype(np.float32)))

    def wtrip(W):
        Wp = 32.0 * np.ascontiguousarray(W, np.float32)
        Wr = Wp.reshape(KO, 128, H).transpose(1, 0, 2)  # [ki, ko, o]
        w8 = f8(Wr)
        dw8 = f8(16.0 * (Wr - w8.astype(np.float32)))
        w8s = f8(Wr / 16.0)
        return w8, dw8, w8s

    wq3 = wtrip(Wq)
    wk3 = wtrip(Wk)
    # packed QK triples: wqk[blk][ki, ko, iw, it, 128]
    wqk = np.empty((KO, 128, KO, 2, 3, 128), F8)
    for blk in range(KO):
        c = slice(blk * 128, blk * 128 + 128)
        for it in range(3):
            wqk[blk, :, :, 0, it, :] = wq3[it][:, :, c]
            wqk[blk, :, :, 1, it, :] = wk3[it][:, :, c]
    wv8, dwv8, swv8 = wtrip(Wv)
    wvres = np.stack([dwv8, swv8], axis=2)  # [128, KO, 2, H]
    wo16 = np.asarray(
        np.ascontiguousarray(Wo, np.float32).reshape(KO, 128, H)
        .transpose(1, 0, 2), BF16)
    common = {"wv8": wv8, "wvres": wvres, "wo": wo16,
              "ident": np.eye(128, dtype=BF16)}
    for b in range(KO):
        common[f"wqk{b}"] = wqk[b]
    return common, x8, xres


def kernel(hidden_states, Wq, bq, Wk, bk, Wv, bv, Wo, bo):
    from concourse.bass_utils import run_bass_kernel_spmd

    zero_bias = not (np.any(bq) or np.any(bk) or np.any(bv) or np.any(bo))
    key = ("nc", zero_bias)
    if key not in _CACHE:
        _CACHE[key] = _build_nc(zero_bias)
    nc = _CACHE[key]

    common, x8, xres = _prep(hidden_states, Wq, Wk, Wv, Wo)
    if not zero_bias:
        common.update({
            "bq32": np.ascontiguousarray(32.0 * bq, np.float32),
            "bk32": np.ascontiguousarray(32.0 * bk, np.float32),
            "bv32": np.asarray(32.0 * bv, BF16).reshape(1, H),
            "bo8": np.asarray(8.0 * bo, BF16).reshape(1, H),
            "ones": np.ones((1, 128), BF16),
        })
    in_maps = [dict(common, x8=x8[b], xres=xres[b]) for b in range(B)]

    res = run_bass_kernel_spmd(nc, in_maps, core_ids=list(range(B)))
    out = np.stack([np.asarray(r["out"]).astype(np.float32)
                    for r in res.results], axis=0)
    return out


if __name__ == "__main__":
    rng = np.random.default_rng(0)
    inputs = {
        "hidden_states": rng.standard_normal((B, S, H)).astype(np.float32),
        "Wq": (rng.standard_normal((H, H)) * 0.02).astype(np.float32),
        "bq": np.zeros(H, np.float32),
        "Wk": (rng.standard_normal((H, H)) * 0.02).astype(np.float32),
        "bk": np.zeros(H, np.float32),
        "Wv": (rng.standard_normal((H, H)) * 0.02).astype(np.float32),
        "bv": np.zeros(H, np.float32),
        "Wo": (rng.standard_normal((H, H)) * 0.02).astype(np.float32),
        "bo": np.zeros(H, np.float32),
    }
    got = kernel(**inputs)
    print("kernel output:", got.shape, got.dtype)
